# revision 7
# baseline (speedup 1.0000x reference)
"""MHA kernel for TRN2: x[8,512,32,32], 8 heads, S=1024, C=512.

Sharding: data-parallel over batch N=8 -> one batch item per NeuronCore.
Per-core layout (all transpose-free, bf16 matmuls, fp32 psum):
  qkT[e,s]  = w_qkvT.T @ x            (e on partitions; w cols host-reordered
                                       [q_p0|k_p0|...|q_p3|k_p3|v])
  v[s,e]    = x.T @ w_v               (s on partitions)
  scoresT   = kT_h.T @ qT_h           (k_s on partitions; head pair packed at
                                       PE rows 0-63 / 64-127)
  P         = exp(scoresT/8)          (ACT, 1024-wide from PSUM)
  oT_aug    = [v_h | 1].T @ P         (M=65; row 64 = softmax denominator r)
  oT        = oT_aug[:64] * (1/r)     (recip + gpsimd partition_broadcast)
  yT[o,s]   = w_outT.T @ oT           (b_out added host-side; y stored bf16)
Schedule: per head-pair step, QK->exp->PV fused; PVs catch up over slots 4-7;
pair p's PV(7) + normalization chain run in step p+1's slot 0; qkv groups for
pair p+1 drain inside step p's slots.
"""

import numpy as np
import ml_dtypes

import concourse.bacc as bacc
import concourse.mybir as mybir
import concourse.tile as tile
from concourse.bass_utils import run_bass_kernel_spmd

P = 128
S = 1024          # sequence = 32*32
C = 512           # channels
NH = 8            # heads
HD = 64           # head dim
CT = C // P       # 4 c-tiles
MT = S // P       # 8 s-tiles
NP = NH // 2      # 4 head pairs
BF = mybir.dt.bfloat16
F32 = mybir.dt.float32
DRAIN = ((0, 0), (1, 0), (0, 1), (1, 1))  # (hh, nt)
PV_AT = {4: (0,), 5: (1, 2), 6: (3, 4), 7: (5, 6)}  # PV(7) -> next step

_cache = {}


def build_program():
    nc = bacc.Bacc("TRN2", target_bir_lowering=False, debug=False, num_devices=8)
    x_d = nc.dram_tensor("x", [C, S], BF, kind="ExternalInput").ap()
    # host-reordered columns: [q_p0|k_p0|q_p1|k_p1|q_p2|k_p2|q_p3|k_p3|v]
    wq_d = nc.dram_tensor("wq", [C, 3 * C], BF, kind="ExternalInput").ap()
    wo_d = nc.dram_tensor("wo", [C, C], BF, kind="ExternalInput").ap()
    y_d = nc.dram_tensor("y", [C, S], BF, kind="ExternalOutput").ap()

    with tile.TileContext(nc) as tc:
        with (
            tc.tile_pool(name="const", bufs=1) as cpool,
            tc.tile_pool(name="qk", bufs=1) as qkpool,
            tc.tile_pool(name="vp", bufs=1) as vpool,
            tc.tile_pool(name="pp", bufs=10) as ppool,
            tc.tile_pool(name="ot", bufs=1) as opool,
            tc.tile_pool(name="yp", bufs=8) as ypool,
            tc.tile_pool(name="misc", bufs=4) as mpool,
            tc.tile_pool(name="psq", bufs=2, space="PSUM") as psq_pool,
            tc.tile_pool(name="pso", bufs=4, space="PSUM") as pso_pool,
        ):
            # ---- load inputs (x first: it gates the first matmul) ----
            # separate tiles per w region so a matmul only waits on the DMA
            # that actually feeds it
            x_sb = cpool.tile([P, CT * S], BF, name="xsb", tag="xsb")
            wp0_sb = cpool.tile([P, CT * 256], BF, name="wp0", tag="wp0")
            wvv_sb = cpool.tile([P, CT * 512], BF, name="wvv", tag="wvv")
            wpr_sb = cpool.tile([P, CT * 768], BF, name="wpr", tag="wpr")
            wo_sb = cpool.tile([P, CT * C], BF, name="wosb", tag="wosb")
            nc.sync.dma_start(
                x_sb.rearrange("p (f s) -> p f s", f=CT),
                x_d.rearrange("(f p) s -> p f s", p=P),
            )
            wqv = wq_d.rearrange("(f p) e -> p f e", p=P)
            nc.sync.dma_start(
                wp0_sb.rearrange("p (f e) -> p f e", f=CT), wqv[:, :, 0:256]
            )
            nc.sync.dma_start(
                wvv_sb.rearrange("p (f e) -> p f e", f=CT), wqv[:, :, 1024:1536]
            )
            nc.sync.dma_start(
                wpr_sb.rearrange("p (f e) -> p f e", f=CT), wqv[:, :, 256:1024]
            )
            nc.sync.dma_start(
                wo_sb.rearrange("p (f e) -> p f e", f=CT),
                wo_d.rearrange("(f p) e -> p f e", p=P),
            )

            def w_slice(col, ct):
                # col indexes the reordered [q_p0|k_p0|...|q_p3|k_p3|v] layout
                if col < 256:
                    return wp0_sb[:, ct * 256 + col:ct * 256 + col + 128]
                if col < 1024:
                    c = col - 256
                    return wpr_sb[:, ct * 768 + c:ct * 768 + c + 128]
                c = col - 1024
                return wvv_sb[:, ct * 512 + c:ct * 512 + c + 512]

            q_sb = [qkpool.tile([P, S], BF, name=f"q{p}", tag=f"q{p}")
                    for p in range(NP)]
            k_sb = [qkpool.tile([P, S], BF, name=f"k{p}", tag=f"k{p}")
                    for p in range(NP)]
            v_sb = [None] * MT
            oT_sb = [opool.tile([P, S], BF, name=f"o{ct}", tag=f"o{ct}")
                     for ct in range(CT)]

            def g_qkv(dst, col, nt, pool, copy_eng=None):
                ps = pool.tile([P, 512], F32, name="gq", tag=pool.name)
                for ct in range(CT):
                    nc.tensor.matmul(
                        ps[:],
                        w_slice(col, ct),
                        x_sb[:, ct * S + nt * 512:ct * S + (nt + 1) * 512],
                        start=(ct == 0), stop=(ct == CT - 1),
                    )
                dst_sl = dst[:, nt * 512:(nt + 1) * 512]
                if copy_eng == "scalar":
                    nc.scalar.copy(dst_sl, ps[:])
                else:
                    nc.vector.tensor_copy(dst_sl, ps[:])

            def g_v(mt, pool):
                ps = pool.tile([P, 512], F32, name="gv", tag=pool.name)
                for ct in range(CT):
                    nc.tensor.matmul(
                        ps[:],
                        x_sb[:, ct * S + mt * P:ct * S + (mt + 1) * P],
                        w_slice(1024, ct),
                        start=(ct == 0), stop=(ct == CT - 1),
                    )
                vt = vpool.tile([P, NH * (HD + 1)], BF, name=f"v{mt}", tag=f"v{mt}")
                vv = vt.rearrange("p (h e) -> p h e", e=HD + 1)
                nc.gpsimd.memset(vv[:, :, HD:HD + 1], 1.0)
                nc.vector.tensor_copy(
                    vv[:, :, 0:HD], ps.rearrange("p (h d) -> p h d", d=HD)
                )
                v_sb[mt] = vt

            def qk_groups(p):
                return [("q", p, 0), ("k", p, 0), ("q", p, 1), ("k", p, 1)]

            drains = {
                0: [("v", m, 0) for m in range(MT)] + qk_groups(1),
                1: qk_groups(2),
                2: qk_groups(3),
                3: [],
            }
            drain_quota = {
                0: [2, 2, 2, 1, 1, 2, 1, 1],
                1: [1, 1, 1, 1, 0, 0, 0, 0],
                2: [1, 1, 1, 1, 0, 0, 0, 0],
                3: [0] * 8,
            }

            def emit_drain(item):
                kind, a, nt = item
                if kind == "v":
                    g_v(a, psq_pool)
                elif kind == "q":
                    g_qkv(q_sb[a], a * 256, nt, psq_pool)
                else:
                    g_qkv(k_sb[a], a * 256 + 128, nt, psq_pool)

            def emit_pv(pp, pso_t, ptiles, m):
                for idx, (hh, nt) in enumerate(DRAIN):
                    h = 2 * pp + hh
                    nc.tensor.matmul(
                        pso_t[idx][0:HD + 1, :],
                        v_sb[m][:, h * (HD + 1):(h + 1) * (HD + 1)],
                        ptiles[(m, nt)][:, hh * 512:(hh + 1) * 512],
                        start=(m == 0), stop=(m == MT - 1),
                    )

            def emit_chain(pp, pso_t, tail=False):
                """normalization: oT[pp] = pso[:64] * (1/pso[64]) per (hh,nt);
                interleaved so muls start as soon as broadcasts land."""
                bc = []
                for idx in range(4):
                    r0 = mpool.tile([1, 512], F32, name="rr", tag="rr")
                    if tail:
                        nc.scalar.copy(r0[0:1, :], pso_t[idx][HD:HD + 1, :])
                    else:
                        nc.vector.tensor_copy(r0[0:1, :], pso_t[idx][HD:HD + 1, :])
                    r1 = mpool.tile([1, 512], F32, name="ri", tag="ri")
                    nc.vector.reciprocal_approx_fast(r1[0:1, :], r0[0:1, :])
                    b0 = mpool.tile([HD, 512], F32, name="bc", tag="bc")
                    nc.gpsimd.partition_broadcast(b0[:], r1[0:1, :], channels=HD)
                    bc.append(b0)
                    if idx >= 1:
                        j = idx - 1
                        hh, nt = DRAIN[j]
                        nc.vector.tensor_mul(
                            oT_sb[pp][hh * HD:(hh + 1) * HD, nt * 512:(nt + 1) * 512],
                            pso_t[j][0:HD, :], bc[j][:],
                        )
                hh, nt = DRAIN[3]
                nc.vector.tensor_mul(
                    oT_sb[pp][hh * HD:(hh + 1) * HD, nt * 512:(nt + 1) * 512],
                    pso_t[3][0:HD, :], bc[3][:],
                )

            # ---- pair-0 projection groups (pso pool is free pre-attention) ----
            g_qkv(q_sb[0], 0, 0, pso_pool)
            g_qkv(k_sb[0], 128, 0, pso_pool, copy_eng="scalar")
            g_qkv(q_sb[0], 0, 1, pso_pool)
            g_qkv(k_sb[0], 128, 1, pso_pool, copy_eng="scalar")

            # ---- attention steps ----
            prev = None  # (p-1, pso_t, ptiles)
            for p in range(NP):
                pso_t = [pso_pool.tile([P, 512], F32, name=f"pso{i}", tag="pso")
                         for i in range(4)]
                ptiles = {}
                dq = list(drains[p])
                quota = drain_quota[p]
                for s in range(MT):
                    for nt in range(2):
                        psq = psq_pool.tile([P, 1024], F32, name="psq", tag="psq")
                        for hh in range(2):
                            nc.tensor.matmul(
                                psq[:, hh * 512:(hh + 1) * 512],
                                k_sb[p][hh * HD:(hh + 1) * HD, s * P:(s + 1) * P],
                                q_sb[p][hh * HD:(hh + 1) * HD, nt * 512:(nt + 1) * 512],
                                start=True, stop=True,
                            )
                        pt = ppool.tile([P, 1024], BF, name="ptile", tag="ptile")
                        nc.scalar.activation(
                            pt[:], psq[:], mybir.ActivationFunctionType.Exp,
                            scale=float(1.0 / np.sqrt(HD)),
                        )
                        ptiles[(s, nt)] = pt
                    if prev is not None and s == 0:
                        emit_pv(prev[0], prev[1], prev[2], 7)
                        emit_chain(prev[0], prev[1])
                    for m in PV_AT.get(s, ()):
                        emit_pv(p, pso_t, ptiles, m)
                    for _ in range(quota[s]):
                        if dq:
                            emit_drain(dq.pop(0))
                prev = (p, pso_t, ptiles)

            # ---- tail: PV(3,7), chain(3), output projection ----
            emit_pv(3, prev[1], prev[2], 7)
            emit_chain(3, prev[1], tail=True)
            # 8 proj groups: first two on the psq ring (free after last exps),
            # then four on the pso ring (WARs stagger behind the muls), last
            # two back on psq (WAR on the first y copies).
            groups = [(0, 0, "psq"), (1, 0, "psq"), (2, 0, "pso"), (3, 0, "pso"),
                      (0, 1, "pso"), (1, 1, "pso"), (2, 1, "psq"), (3, 1, "psq")]
            for ot, st, ring in groups:
                pool = psq_pool if ring == "psq" else pso_pool
                ps = pool.tile([P, 512], F32, name="op", tag=pool.name if ring == "psq" else "pso")
                for ct in range(CT):
                    nc.tensor.matmul(
                        ps[:],
                        wo_sb[:, ct * 512 + ot * P:ct * 512 + (ot + 1) * P],
                        oT_sb[ct][:, st * 512:(st + 1) * 512],
                        start=(ct == 0), stop=(ct == CT - 1),
                    )
                yt = ypool.tile([P, 512], BF, name="yt", tag="yt")
                nc.scalar.activation(
                    yt[:], ps[:], mybir.ActivationFunctionType.Copy
                )
                nc.sync.dma_start(
                    y_d[ot * P:(ot + 1) * P, st * 512:(st + 1) * 512], yt[:]
                )

    nc.compile()
    return nc


def get_program():
    if "nc" not in _cache:
        _cache["nc"] = build_program()
    return _cache["nc"]


_COL_ORDER = np.concatenate(
    [np.r_[p * 128:(p + 1) * 128, 512 + p * 128:512 + (p + 1) * 128]
     for p in range(NP)] + [np.r_[1024:1536]]
)


def kernel(x, w_qkv, w_out, b_out, _trace=False, _tmpdir=None):
    x = np.asarray(x, dtype=np.float32)
    w_qkv = np.asarray(w_qkv, dtype=np.float32)
    w_out = np.asarray(w_out, dtype=np.float32)
    b_out = np.asarray(b_out, dtype=np.float32)
    N = x.shape[0]

    xb = x.reshape(N, C, S).astype(ml_dtypes.bfloat16)
    wqT = np.ascontiguousarray(w_qkv.T[:, _COL_ORDER]).astype(ml_dtypes.bfloat16)
    woT = np.ascontiguousarray(w_out.T).astype(ml_dtypes.bfloat16)

    nc = get_program()
    in_maps = [
        {"x": np.ascontiguousarray(xb[n]), "wq": wqT, "wo": woT}
        for n in range(N)
    ]
    res = run_bass_kernel_spmd(
        nc, in_maps, core_ids=list(range(N)), trace=_trace, tmpdir=_tmpdir
    )
    y = np.stack([res.results[n]["y"] for n in range(N)]).astype(np.float32)
    y = y.reshape(N, C, 32, 32)
    y = y + b_out[None, :, None, None]
    if _trace:
        return y, res
    return y
